# revision 30
# baseline (speedup 1.0000x reference)
"""Trainium2 Bass kernel for the CapsuleNetwork routing problem.

Problem (per reference):
  B, L, D, K = 1024, 200, 64, 4 ; E = K*D = 256
  hat[b,l,e] = sum_d seq[b,l,d] * W[l,e,d]          (einsum, PE)
  3 rounds of dynamic routing over interests K (softmax over K per (b,l)),
  cap = squash(w @ hat), cw += hat . cap            (DVE/Pool)
  output cap -> [B, K, D]

Sharding: pure data-parallel over batch across 8 NeuronCores (128 rows each);
weights replicated. Host-side layout transforms; bf16 input DMAs.

v2 design ("F2"):
  - hat is computed ONCE on the PE (bf16 inputs) and stored bf16 in SBUF
    [128, L, E]; all three routing iterations read it from SBUF. This kills
    2/3 of the PE matmuls and 2/3 of the PSUM->SBUF copies vs recomputing.
  - Elementwise routing work is column-split between DVE (bf16 2x perf mode,
    0.521 ns/elem) and the otherwise-idle Pool engine (0.833 ns/elem):
    DVE takes d-columns [0, DS), Pool [DS, D). Reductions are pairwise
    tree-folds (general widths) on the same engines.
  - Phase 1 streams: DMA chunk -> PE matmuls (parity-grouped PSUM tiles) ->
    ACT/Pool copies to hat -> first-iteration capacc, all pipelined.
  - Streams 2/3 are fused per 16-l chunk: delta (hat . cap) -> cw update ->
    softmax -> capacc (w * hat, folded into capRaw).

Restructured routing algebra (same as v1, validated vs reference):
  cw layout [B, L, K];   w = exp(cw) / sum_k exp(cw)
  capRaw[b,:,k] = sum_l w[b,l,k] hat[b,l,:,k]
  n = |capRaw|^2 ; s = n/(1+n)/sqrt(n+1e-9)
  cw += s[b,k] * (hat . capRaw)    (squash scale folded into the cw update)
  final out = s * capRaw
"""

import os
import sys

import numpy as np

for _p in ("/opt/trn_rl_repo", "/root/.axon_site/_ro/trn_rl_repo"):
    if os.path.isdir(_p) and _p not in sys.path:
        sys.path.insert(0, _p)

B, L, D, K = 1024, 200, 64, 4
E = K * D
NCORES = 8
BS = B // NCORES  # 128 batch rows per core
M = L // 2        # l-pairs: partition p = (l%2)*64 + d'

MC = 8            # m's per phase-1 chunk (16 l's)
PB = int(os.environ.get("KERNEL_PB", "20"))  # l's per routing chunk in streams 2/3
LPB = 512 // E    # l's per PSUM bank (2)
LH = 100          # l's per half (l = ch*LH + lp; lp is the Aw partition axis)
NDC = 4           # d' columns per Aw chunk

# column splits (in d units; columns are (d, k) pairs, k innermost)
DS_DEL = int(os.environ.get("KERNEL_DS_DEL", "36"))   # DVE share for delta
DS_CP = int(os.environ.get("KERNEL_DS_CP", "48"))     # ACT share of hat copies (rest DVE; Pool can't read PSUM)
# Aw chunk indices handled by Pool (rest DVE); 6/16 ~ the Pool/DVE speed ratio
AW_POOL = frozenset((2, 5, 7, 10, 13, 15))


def build_nc():
    """Build the Bass program for one core (SPMD; all cores run the same NEFF)."""
    import concourse.bass as bass
    import concourse.tile as tile
    from concourse import bacc, mybir

    f32 = mybir.dt.float32
    bf16 = mybir.dt.bfloat16
    AF = mybir.ActivationFunctionType
    OP = mybir.AluOpType

    from concourse import masks

    nc = bacc.Bacc(trn_type="TRN2", target_bir_lowering=False, debug=False)
    seqT_d = nc.dram_tensor("seqT", [128, M, BS], bf16, kind="ExternalInput")
    wT_d = nc.dram_tensor("wT", [128, M, E], bf16, kind="ExternalInput")
    seqL_d = nc.dram_tensor("seqL", [LH, 2, D, BS], bf16, kind="ExternalInput")
    w5_d = nc.dram_tensor("w5", [LH, 2, D, E], bf16, kind="ExternalInput")
    cw_d = nc.dram_tensor("cw", [BS, L, K], f32, kind="ExternalInput")
    out_d = nc.dram_tensor("out", [BS, E], f32, kind="ExternalOutput")
    DEBUG = os.environ.get("KERNEL_DEBUG", "0") == "1"
    if DEBUG:
        dbg1_d = nc.dram_tensor("dbg1", [BS, D, K], bf16, kind="ExternalOutput")
        dbg2_d = nc.dram_tensor("dbg2", [BS, E], f32, kind="ExternalOutput")
        dbg3_d = nc.dram_tensor("dbg3", [100, 2, K, BS], bf16, kind="ExternalOutput")

    NMC = (M + MC - 1) // MC

    with tile.TileContext(nc) as tc:
        with (
            tc.tile_pool(name="consts", bufs=1) as consts,
            tc.tile_pool(name="seqp", bufs=2) as seqp,
            tc.tile_pool(name="wtp", bufs=2) as wtp,
            tc.tile_pool(name="hatps", bufs=2, space="PSUM") as psum,
            tc.tile_pool(name="scr", bufs=2) as scr,
            tc.tile_pool(name="w5p", bufs=2) as w5p,
            tc.tile_pool(name="awp", bufs=2) as awp,
            tc.tile_pool(name="trp", bufs=2, space="PSUM") as trp,
            tc.tile_pool(name="capps", bufs=1, space="PSUM") as capps,
        ):
            hat = consts.tile([BS, L, E], bf16, name="hat_sb")
            cw = consts.tile([BS, L, K], f32, name="cw_sb")
            w = consts.tile([BS, L, K], f32, name="w_sb")
            zsum = consts.tile([BS, L], f32, name="zsum")
            zinv = consts.tile([BS, L], f32, name="zinv")
            wB = consts.tile([BS, L, K], bf16, name="wB_sb")
            capB = consts.tile([BS, D, K], bf16, name="capB")
            capRaw = consts.tile([BS, D, K], f32, name="capRaw")
            capOut = consts.tile([BS, E], f32, name="capOut")
            seqL = consts.tile([LH, 2, D, BS], bf16, name="seqL_sb")
            wTs = consts.tile([LH, 2, K, BS], bf16, name="wTs_sb")
            ident = consts.tile([128, 128], bf16, name="ident")
            masks.make_identity(nc, ident[:])
            smalls = consts.tile([BS, 8, K], f32, name="smalls")
            nvec = smalls[:, 0, :]
            lnt = smalls[:, 1, :]
            rt = smalls[:, 2, :]
            np1 = smalls[:, 3, :]
            den = smalls[:, 4, :]
            dinv = smalls[:, 5, :]
            svec = smalls[:, 6, :]
            epsB = consts.tile([BS, 1], f32, name="epsB")
            nc.vector.memset(epsB[:], 1e-9)

            nc.sync.dma_start(out=cw[:], in_=cw_d[:])

            def softmax_full():
                nc.scalar.activation(out=w[:], in_=cw[:], func=AF.Exp)
                nc.vector.tensor_reduce(
                    out=zsum[:], in_=w[:], axis=mybir.AxisListType.X, op=OP.add
                )
                nc.vector.reciprocal(out=zinv[:], in_=zsum[:])
                zin = bass.AP(
                    tensor=zinv.tensor,
                    offset=zinv.offset,
                    ap=[zinv.ap[0], [1, L], [0, K]],
                )
                nc.vector.tensor_tensor(
                    out=wB[:], in0=w[:], in1=zin, op=OP.mult
                )

            def fold_l(eng, u, nl, c0, c1):
                """Pairwise-fold u[:, 0:nl, c0:c1] over the l axis into u[:,0,:]."""
                width = nl
                while width > 1:
                    h = width // 2
                    eng.tensor_add(
                        out=u[:, 0:h, c0:c1],
                        in0=u[:, 0:h, c0:c1],
                        in1=u[:, width - h : width, c0:c1],
                    )
                    width -= h

            def fold_d(eng, u, nl, nd):
                """Fold u[:, 0:nl, 0:nd*K] over d (d-major slabs of K cols)."""
                width = nd
                while width > 1:
                    h = width // 2
                    eng.tensor_add(
                        out=u[:, 0:nl, 0 : h * K],
                        in0=u[:, 0:nl, 0 : h * K],
                        in1=u[:, 0:nl, (width - h) * K : width * K],
                    )
                    width -= h

            def capacc_chunk(l0, nl):
                """capRaw[:, cols] += sum_l w[b,l,k]*hat[b,l,cols] for this chunk,
                column-split DVE [0, DS_DEL) / Pool [DS_DEL, D). Phase-1 only;
                shares scratch tags with delta_cw_chunk."""
                for eng, d0, d1, tag in (
                    (nc.vector, 0, DS_DEL, "ud_d"),
                    (nc.gpsimd, DS_DEL, D, "ud_p"),
                ):
                    ncol = (d1 - d0) * K
                    u = scr.tile([BS, PB, ncol], bf16, name=tag, tag=tag)
                    win = bass.AP(
                        tensor=wB.tensor,
                        offset=wB.offset + l0 * K,
                        ap=[wB.ap[0], [K, nl], [0, d1 - d0], [1, K]],
                    )
                    eng.tensor_tensor(
                        out=u[:, 0:nl, :],
                        in0=hat[:, l0 : l0 + nl, d0 * K : d1 * K],
                        in1=win,
                        op=OP.mult,
                    )
                    fold_l(eng, u, nl, 0, ncol)
                    eng.tensor_add(
                        out=capRaw[:, d0:d1, :],
                        in0=capRaw[:, d0:d1, :],
                        in1=u[:, 0, :],
                    )

            def transposes_half(ch):
                """wTs[lp, ch, k, b] = wB[b, ch*LH+lp, k] for one l half
                (PE transpose via identity, ACT copy out of PSUM; the [LH, BS]
                shapes keep every partition access base-0)."""
                for k in range(K):
                    tp = trp.tile([LH, BS], bf16, name="tp", tag="tp")
                    nc.tensor.transpose(
                        tp[:, :], wB[:, ch * LH : (ch + 1) * LH, k], ident[:, :]
                    )
                    nc.scalar.copy(out=wTs[:, ch, k, :], in_=tp[:, :])

            def aw_phase(ps2):
                """capRaw(PSUM)[b, k, d] = sum_{l,d'} w[b,l,k] seq[b,l,d']
                W[l, kD+d, d'].  Aw = w*seq formed on DVE/Pool in l-partition
                layout; PE contracts (l, d') into 4 per-k PSUM chains."""
                nch = D // NDC
                for ci in range(nch):
                    dc = ci * NDC
                    w5c = w5p.tile([LH, 2, NDC, E], bf16, name="w5c", tag="w5")
                    nc.sync.dma_start(
                        out=w5c[:], in_=w5_d[:, :, dc : dc + NDC, :]
                    )
                    aw = awp.tile([LH, 2, NDC, K, BS], bf16, name="aw", tag="aw")
                    eng = nc.gpsimd if ci in AW_POOL else nc.vector
                    for ch in (0, 1):
                        seqin = bass.AP(
                            tensor=seqL.tensor,
                            offset=seqL.offset + (ch * D + dc) * BS,
                            ap=[seqL.ap[0], [BS, NDC], [0, K], [1, BS]],
                        )
                        win = bass.AP(
                            tensor=wTs.tensor,
                            offset=wTs.offset + ch * K * BS,
                            ap=[wTs.ap[0], [0, NDC], [BS, K], [1, BS]],
                        )
                        eng.tensor_tensor(
                            out=aw[:, ch], in0=seqin, in1=win, op=OP.mult
                        )
                    for ch in (0, 1):
                        for j in range(NDC):
                            for k in range(K):
                                nc.tensor.matmul(
                                    ps2[:, k, :],
                                    lhsT=aw[:, ch, j, k, :],
                                    rhs=w5c[:, ch, j, k * D : (k + 1) * D],
                                    # ps2 is 1KB/partition: all four k chains
                                    # share one 2KB PSUM zero region, so only
                                    # the very first matmul may set start.
                                    start=(
                                        ci == 0 and ch == 0 and j == 0 and k == 0
                                    ),
                                    stop=(
                                        ci == nch - 1 and ch == 1 and j == NDC - 1
                                    ),
                                    skip_group_check=True,
                                )

            def squash_psum(ps2, final):
                """squash scalars from the PSUM capRaw; emit capB (pre-scaled
                cap for the next delta) or the final output."""
                capF = scr.tile([BS, K, D], f32, name="capF", tag="capF")
                nc.vector.tensor_copy(out=capF[:], in_=ps2[:])
                for k in range(K):
                    u2 = scr.tile([BS, D], f32, name="u2", tag="u2")
                    nc.vector.scalar_tensor_tensor(
                        out=u2[:],
                        in0=capF[:, k, :],
                        scalar=1.0,
                        in1=capF[:, k, :],
                        op0=OP.mult,
                        op1=OP.mult,
                        accum_out=nvec[:, k : k + 1],
                    )
                nc.scalar.activation(out=lnt, in_=nvec, func=AF.Ln, bias=epsB[:])
                nc.scalar.activation(out=rt, in_=lnt, func=AF.Exp, scale=0.5)
                nc.vector.tensor_scalar_add(out=np1, in0=nvec, scalar1=1.0)
                nc.vector.tensor_mul(out=den, in0=np1, in1=rt)
                nc.vector.reciprocal(out=dinv, in_=den)
                nc.vector.tensor_mul(out=svec, in0=nvec, in1=dinv)
                for k in range(K):
                    if final:
                        nc.vector.tensor_scalar_mul(
                            out=capOut[:, k * D : (k + 1) * D],
                            in0=capF[:, k, :],
                            scalar1=svec[:, k : k + 1],
                        )
                    else:
                        nc.vector.tensor_scalar_mul(
                            out=capB[:, :, k],
                            in0=capF[:, k, :],
                            scalar1=svec[:, k : k + 1],
                        )

            def delta_cw_chunk(l0, nl):
                """cw[:, l0:l0+nl, :] += sum_d hat[b,l,d,k]*capB[b,d,k].
                capB is pre-scaled by svec (squash), so this IS the cw update.
                Column-split DVE/Pool; both partials added into cw."""
                parts = []
                for eng, d0, d1, tag in (
                    (nc.vector, 0, DS_DEL, "ud_d"),
                    (nc.gpsimd, DS_DEL, D, "ud_p"),
                ):
                    nd = d1 - d0
                    u = scr.tile([BS, PB, nd * K], bf16, name=tag, tag=tag)
                    cin = bass.AP(
                        tensor=capB.tensor,
                        offset=capB.offset + d0 * K,
                        ap=[capB.ap[0], [0, nl], [1, nd * K]],
                    )
                    eng.tensor_tensor(
                        out=u[:, 0:nl, :],
                        in0=hat[:, l0 : l0 + nl, d0 * K : d1 * K],
                        in1=cin,
                        op=OP.mult,
                    )
                    fold_d(eng, u, nl, nd)
                    parts.append(u)
                for u in parts:
                    nc.vector.tensor_add(
                        out=cw[:, l0 : l0 + nl, :],
                        in0=cw[:, l0 : l0 + nl, :],
                        in1=u[:, 0:nl, 0:K],
                    )

            def softmax_chunk(l0, nl):
                """wB[:, l0:l0+nl, :] = softmax_k(cw) in bf16. w holds exp(cw);
                the normalize + bf16 downcast are fused into one mult."""
                nc.scalar.activation(
                    out=w[:, l0 : l0 + nl, :], in_=cw[:, l0 : l0 + nl, :], func=AF.Exp
                )
                nc.vector.tensor_reduce(
                    out=zsum[:, l0 : l0 + nl],
                    in_=w[:, l0 : l0 + nl, :],
                    axis=mybir.AxisListType.X,
                    op=OP.add,
                )
                nc.vector.reciprocal(
                    out=zinv[:, l0 : l0 + nl], in_=zsum[:, l0 : l0 + nl]
                )
                zin = bass.AP(
                    tensor=zinv.tensor,
                    offset=zinv.offset + l0,
                    ap=[zinv.ap[0], [1, nl], [0, K]],
                )
                nc.vector.tensor_tensor(
                    out=wB[:, l0 : l0 + nl, :],
                    in0=w[:, l0 : l0 + nl, :],
                    in1=zin,
                    op=OP.mult,
                )

            def squash_scalars():
                for k in range(K):
                    u2 = scr.tile([BS, D], f32, name="u2", tag="u2")
                    nc.vector.scalar_tensor_tensor(
                        out=u2[:],
                        in0=capRaw[:, :, k],
                        scalar=1.0,
                        in1=capRaw[:, :, k],
                        op0=OP.mult,
                        op1=OP.mult,
                        accum_out=nvec[:, k : k + 1],
                    )
                nc.scalar.activation(out=lnt, in_=nvec, func=AF.Ln, bias=epsB[:])
                nc.scalar.activation(out=rt, in_=lnt, func=AF.Exp, scale=0.5)
                nc.vector.tensor_scalar_add(out=np1, in0=nvec, scalar1=1.0)
                nc.vector.tensor_mul(out=den, in0=np1, in1=rt)
                nc.vector.reciprocal(out=dinv, in_=den)
                nc.vector.tensor_mul(out=svec, in0=nvec, in1=dinv)
                # capB = svec * capRaw (pre-scaled by the squash factor, so the
                # delta contraction directly yields the cw increment)
                sin = bass.AP(
                    tensor=smalls.tensor,
                    offset=smalls.offset + 6 * K,
                    ap=[smalls.ap[0], [0, D], [1, K]],
                )
                nc.vector.tensor_tensor(
                    out=capB[:], in0=capRaw[:], in1=sin, op=OP.mult
                )

            # ================= phase 1: hat einsum + first capacc =================
            softmax_full()
            nc.vector.memset(capRaw[:], 0.0)
            nc.sync.dma_start(out=seqL[:], in_=seqL_d[:])

            for ci in range(NMC):
                mc = ci * MC
                nm = min(MC, M - mc)
                seqc = seqp.tile([128, MC, BS], bf16, name="seqc", tag="seqc")
                wc = wtp.tile([128, MC, E], bf16, name="wc", tag="wc")
                nc.sync.dma_start(
                    out=seqc[:, 0:nm, :], in_=seqT_d[:, mc : mc + nm, :]
                )
                nc.sync.dma_start(out=wc[:, 0:nm, :], in_=wT_d[:, mc : mc + nm, :])
                # parity-grouped matmuls: all even l's of the chunk, then odd
                for par in (0, 1):
                    p0 = 64 * par
                    for b0 in range(0, nm, 4):
                        nb = min(4, nm - b0)
                        ps = psum.tile([128, 4, E], f32, name="ps", tag="ps")
                        for j in range(nb):
                            mi = b0 + j
                            nc.tensor.matmul(
                                ps[:, j, :],
                                lhsT=seqc[p0 : p0 + 64, mi, :],
                                rhs=wc[p0 : p0 + 64, mi, :],
                                start=(j % LPB == 0),
                                stop=(j % LPB == LPB - 1 or j == nb - 1),
                                skip_group_check=True,
                            )
                        # copy to hat rows l = 2*(mc+b0+j) + par, split ACT/Pool
                        hout_a = bass.AP(
                            tensor=hat.tensor,
                            offset=hat.offset + (2 * (mc + b0) + par) * E,
                            ap=[hat.ap[0], [2 * E, nb], [1, DS_CP * K]],
                        )
                        hout_p = bass.AP(
                            tensor=hat.tensor,
                            offset=hat.offset + (2 * (mc + b0) + par) * E + DS_CP * K,
                            ap=[hat.ap[0], [2 * E, nb], [1, (D - DS_CP) * K]],
                        )
                        nc.scalar.copy(out=hout_a, in_=ps[:, 0:nb, 0 : DS_CP * K])
                        nc.vector.tensor_copy(
                            out=hout_p, in_=ps[:, 0:nb, DS_CP * K : E]
                        )
                # first-iteration capacc on the now-complete contiguous l range
                capacc_chunk(2 * mc, 2 * nm)

            squash_scalars()

            # ================= streams 2/3 =================
            if DEBUG:
                nc.sync.dma_start(out=dbg1_d[:], in_=capB[:])

            for it in (1, 2):
                for l0 in range(0, L, PB):
                    nl = min(PB, L - l0)
                    delta_cw_chunk(l0, nl)
                    softmax_chunk(l0, nl)
                    if l0 + nl in (LH, L):
                        transposes_half((l0 + nl) // LH - 1)
                if DEBUG and it == 1:
                    nc.sync.dma_start(out=dbg3_d[:], in_=wTs[:])
                ps2 = capps.tile([BS, K, D], f32, name="ps2", tag="ps2")
                aw_phase(ps2)
                if DEBUG and it == 1:
                    nc.vector.tensor_copy(
                        out=capOut.rearrange("b (k d) -> b k d", k=K), in_=ps2[:]
                    )
                    nc.sync.dma_start(out=dbg2_d[:], in_=capOut[:])
                squash_psum(ps2, final=(it == 2))

            nc.sync.dma_start(out=out_d[:], in_=capOut[:])

    nc.finalize()
    return nc


_NC_CACHE = None


def _get_nc():
    global _NC_CACHE
    if _NC_CACHE is None:
        _NC_CACHE = build_nc()
    return _NC_CACHE


def prep_inputs(seq_out, weights, capsule_weight):
    """Host-side layout prep -> list of per-core input maps."""
    import ml_dtypes

    bf16 = ml_dtypes.bfloat16
    seq = np.ascontiguousarray(np.asarray(seq_out, dtype=np.float32))
    W = np.ascontiguousarray(np.asarray(weights, dtype=np.float32))[0]  # [L,E,D]
    cwf = np.ascontiguousarray(np.asarray(capsule_weight, dtype=np.float32))

    # seqT[p, m, b] = seq[b, 2m + p//64, p%64]
    seqT = np.ascontiguousarray(
        seq.reshape(B, M, 2, D).transpose(2, 3, 1, 0).reshape(128, M, B).astype(bf16)
    )
    # wT[p, m, (d,k)] = W[2m + p//64, k*D + d, p%64]   (hat free axis = (d,k))
    wTf = W.reshape(M, 2, K, D, D).transpose(1, 4, 0, 3, 2)  # [par, d', m, d, k]
    wT = np.ascontiguousarray(wTf.reshape(128, M, E).astype(bf16))
    # cwA[b, l, k] = cw[b, k, l]
    cwA = np.ascontiguousarray(cwf.transpose(0, 2, 1))  # [B, L, K]
    # seqL[lp, ch, d', b] = seq[b, ch*LH+lp, d']
    seqL = np.ascontiguousarray(
        seq.reshape(B, 2, LH, D).transpose(2, 1, 3, 0).astype(bf16)
    )
    # w5[lp, ch, d', e] = W[ch*LH+lp, e, d']
    w5 = np.ascontiguousarray(
        W.reshape(2, LH, E, D).transpose(1, 0, 3, 2).astype(bf16)
    )

    in_maps = []
    for c in range(NCORES):
        in_maps.append(
            {
                "seqT": np.ascontiguousarray(seqT[:, :, c * BS : (c + 1) * BS]),
                "wT": wT,
                "seqL": np.ascontiguousarray(seqL[:, :, :, c * BS : (c + 1) * BS]),
                "w5": w5,
                "cw": np.ascontiguousarray(cwA[c * BS : (c + 1) * BS]),
            }
        )
    return in_maps


def gather_out(results):
    """Per-core 'out' [BS, E=(k*D+d)] -> full [B, K, D]."""
    return np.concatenate(
        [r["out"].reshape(BS, K, D) for r in results], axis=0
    ).astype(np.float32)


def kernel(seq_out, mask, weights, capsule_weight):
    from concourse.bass_utils import run_bass_kernel_spmd

    nc = _get_nc()
    in_maps = prep_inputs(seq_out, weights, capsule_weight)
    res = run_bass_kernel_spmd(nc, in_maps, core_ids=list(range(NCORES)))
    return gather_out(res.results)


if __name__ == "__main__":
    rng = np.random.default_rng(0)
    seq_out = rng.standard_normal((B, L, D), dtype=np.float32)
    mask = np.ones((B, L), dtype=np.float32)
    weights = (0.02 * rng.standard_normal((1, L, E, D))).astype(np.float32)
    capsule_weight = rng.standard_normal((B, K, L), dtype=np.float32)
    out = kernel(seq_out, mask, weights, capsule_weight)
    print("out", out.shape, out.dtype, float(np.abs(out).max()))


# revision 33
# speedup vs baseline: 1.0099x; 1.0099x over previous
"""Trainium2 Bass kernel for the CapsuleNetwork routing problem.

Problem (per reference):
  B, L, D, K = 1024, 200, 64, 4 ; E = K*D = 256
  hat[b,l,e] = sum_d seq[b,l,d] * W[l,e,d]          (einsum, PE)
  3 rounds of dynamic routing over interests K (softmax over K per (b,l)),
  cap = squash(w @ hat), cw += hat . cap            (DVE/Pool)
  output cap -> [B, K, D]

Sharding: pure data-parallel over batch across 8 NeuronCores (128 rows each);
weights replicated. Host-side layout transforms; bf16 input DMAs.

v2 design ("F2"):
  - hat is computed ONCE on the PE (bf16 inputs) and stored bf16 in SBUF
    [128, L, E]; all three routing iterations read it from SBUF. This kills
    2/3 of the PE matmuls and 2/3 of the PSUM->SBUF copies vs recomputing.
  - Elementwise routing work is column-split between DVE (bf16 2x perf mode,
    0.521 ns/elem) and the otherwise-idle Pool engine (0.833 ns/elem):
    DVE takes d-columns [0, DS), Pool [DS, D). Reductions are pairwise
    tree-folds (general widths) on the same engines.
  - Phase 1 streams: DMA chunk -> PE matmuls (parity-grouped PSUM tiles) ->
    ACT/Pool copies to hat -> first-iteration capacc, all pipelined.
  - Streams 2/3 are fused per 16-l chunk: delta (hat . cap) -> cw update ->
    softmax -> capacc (w * hat, folded into capRaw).

Restructured routing algebra (same as v1, validated vs reference):
  cw layout [B, L, K];   w = exp(cw) / sum_k exp(cw)
  capRaw[b,:,k] = sum_l w[b,l,k] hat[b,l,:,k]
  n = |capRaw|^2 ; s = n/(1+n)/sqrt(n+1e-9)
  cw += s[b,k] * (hat . capRaw)    (squash scale folded into the cw update)
  final out = s * capRaw
"""

import os
import sys

import numpy as np

for _p in ("/opt/trn_rl_repo", "/root/.axon_site/_ro/trn_rl_repo"):
    if os.path.isdir(_p) and _p not in sys.path:
        sys.path.insert(0, _p)

B, L, D, K = 1024, 200, 64, 4
E = K * D
NCORES = 8
BS = B // NCORES  # 128 batch rows per core
M = L // 2        # l-pairs: partition p = (l%2)*64 + d'

MC = 8            # m's per phase-1 chunk (16 l's)
PB = int(os.environ.get("KERNEL_PB", "20"))  # l's per routing chunk in streams 2/3
LPB = 512 // E    # l's per PSUM bank (2)
LH = 100          # l's per half (l = ch*LH + lp; lp is the Aw partition axis)
NDC = 4           # d' columns per Aw chunk

# column splits (in d units; columns are (d, k) pairs, k innermost)
DS_DEL = int(os.environ.get("KERNEL_DS_DEL", "36"))   # DVE share for delta
DS_CP = int(os.environ.get("KERNEL_DS_CP", "48"))     # ACT share of hat copies (rest DVE; Pool can't read PSUM)
# Aw chunk indices handled by Pool (rest DVE); 6/16 ~ the Pool/DVE speed ratio
AW_POOL = frozenset((2, 5, 7, 10, 13, 15))


def build_nc():
    """Build the Bass program for one core (SPMD; all cores run the same NEFF)."""
    import concourse.bass as bass
    import concourse.tile as tile
    from concourse import bacc, mybir

    f32 = mybir.dt.float32
    bf16 = mybir.dt.bfloat16
    AF = mybir.ActivationFunctionType
    OP = mybir.AluOpType

    from concourse import masks

    nc = bacc.Bacc(trn_type="TRN2", target_bir_lowering=False, debug=False)
    seqT_d = nc.dram_tensor("seqT", [128, M, BS], bf16, kind="ExternalInput")
    wT_d = nc.dram_tensor("wT", [128, M, E], bf16, kind="ExternalInput")
    seqL_d = nc.dram_tensor("seqL", [LH, 2, D, BS], bf16, kind="ExternalInput")
    w5_d = nc.dram_tensor("w5", [LH, 2, D, E], bf16, kind="ExternalInput")
    cw_d = nc.dram_tensor("cw", [BS, L, K], f32, kind="ExternalInput")
    out_d = nc.dram_tensor("out", [BS, E], f32, kind="ExternalOutput")
    DEBUG = os.environ.get("KERNEL_DEBUG", "0") == "1"
    if DEBUG:
        dbg1_d = nc.dram_tensor("dbg1", [BS, D, K], bf16, kind="ExternalOutput")
        dbg2_d = nc.dram_tensor("dbg2", [BS, E], f32, kind="ExternalOutput")
        dbg3_d = nc.dram_tensor("dbg3", [100, 2, K, BS], bf16, kind="ExternalOutput")

    NMC = (M + MC - 1) // MC

    with tile.TileContext(nc) as tc:
        with (
            tc.tile_pool(name="consts", bufs=1) as consts,
            tc.tile_pool(name="seqp", bufs=2) as seqp,
            tc.tile_pool(name="wtp", bufs=2) as wtp,
            tc.tile_pool(name="hatps", bufs=2, space="PSUM") as psum,
            tc.tile_pool(name="scr", bufs=2) as scr,
            tc.tile_pool(name="w5p", bufs=2) as w5p,
            tc.tile_pool(name="awp", bufs=2) as awp,
            tc.tile_pool(name="trp", bufs=2, space="PSUM") as trp,
            tc.tile_pool(name="capps", bufs=1, space="PSUM") as capps,
        ):
            hat = consts.tile([BS, L, E], bf16, name="hat_sb")
            cw = consts.tile([BS, L, K], f32, name="cw_sb")
            w = consts.tile([BS, L, K], f32, name="w_sb")
            zsum = consts.tile([BS, L], f32, name="zsum")
            zinv = consts.tile([BS, L], f32, name="zinv")
            wB = consts.tile([BS, L, K], bf16, name="wB_sb")
            capB = consts.tile([BS, D, K], bf16, name="capB")
            capRaw = consts.tile([BS, D, K], f32, name="capRaw")
            capOut = consts.tile([BS, E], f32, name="capOut")
            seqL = consts.tile([LH, 2, D, BS], bf16, name="seqL_sb")
            wTs = consts.tile([LH, 2, K, BS], bf16, name="wTs_sb")
            ident = consts.tile([128, 128], bf16, name="ident")
            masks.make_identity(nc, ident[:])
            smalls = consts.tile([BS, 8, K], f32, name="smalls")
            nvec = smalls[:, 0, :]
            lnt = smalls[:, 1, :]
            rt = smalls[:, 2, :]
            np1 = smalls[:, 3, :]
            den = smalls[:, 4, :]
            dinv = smalls[:, 5, :]
            svec = smalls[:, 6, :]
            epsB = consts.tile([BS, 1], f32, name="epsB")
            nc.vector.memset(epsB[:], 1e-9)

            nc.sync.dma_start(out=cw[:], in_=cw_d[:])

            def softmax_full():
                nc.scalar.activation(out=w[:], in_=cw[:], func=AF.Exp)
                nc.vector.tensor_reduce(
                    out=zsum[:], in_=w[:], axis=mybir.AxisListType.X, op=OP.add
                )
                nc.vector.reciprocal(out=zinv[:], in_=zsum[:])
                zin = bass.AP(
                    tensor=zinv.tensor,
                    offset=zinv.offset,
                    ap=[zinv.ap[0], [1, L], [0, K]],
                )
                nc.vector.tensor_tensor(
                    out=wB[:], in0=w[:], in1=zin, op=OP.mult
                )

            def fold_l(eng, u, nl, c0, c1):
                """Pairwise-fold u[:, 0:nl, c0:c1] over the l axis into u[:,0,:]."""
                width = nl
                while width > 1:
                    h = width // 2
                    eng.tensor_add(
                        out=u[:, 0:h, c0:c1],
                        in0=u[:, 0:h, c0:c1],
                        in1=u[:, width - h : width, c0:c1],
                    )
                    width -= h

            def fold_d(eng, u, nl, nd):
                """Fold u[:, 0:nl, 0:nd*K] over d (d-major slabs of K cols)."""
                width = nd
                while width > 1:
                    h = width // 2
                    eng.tensor_add(
                        out=u[:, 0:nl, 0 : h * K],
                        in0=u[:, 0:nl, 0 : h * K],
                        in1=u[:, 0:nl, (width - h) * K : width * K],
                    )
                    width -= h

            def capacc_chunk(l0, nl):
                """capRaw[:, cols] += sum_l w[b,l,k]*hat[b,l,cols] for this chunk,
                column-split DVE [0, DS_DEL) / Pool [DS_DEL, D). Phase-1 only;
                shares scratch tags with delta_cw_chunk."""
                for eng, d0, d1, tag in (
                    (nc.vector, 0, DS_DEL, "ud_d"),
                    (nc.gpsimd, DS_DEL, D, "ud_p"),
                ):
                    ncol = (d1 - d0) * K
                    u = scr.tile([BS, PB, ncol], bf16, name=tag, tag=tag)
                    win = bass.AP(
                        tensor=wB.tensor,
                        offset=wB.offset + l0 * K,
                        ap=[wB.ap[0], [K, nl], [0, d1 - d0], [1, K]],
                    )
                    eng.tensor_tensor(
                        out=u[:, 0:nl, :],
                        in0=hat[:, l0 : l0 + nl, d0 * K : d1 * K],
                        in1=win,
                        op=OP.mult,
                    )
                    fold_l(eng, u, nl, 0, ncol)
                    eng.tensor_add(
                        out=capRaw[:, d0:d1, :],
                        in0=capRaw[:, d0:d1, :],
                        in1=u[:, 0, :],
                    )

            def transposes_half(ch):
                """wTs[lp, ch, k, b] = wB[b, ch*LH+lp, k] for one l half
                (PE transpose via identity, ACT copy out of PSUM; the [LH, BS]
                shapes keep every partition access base-0)."""
                for k in range(K):
                    tp = trp.tile([LH, BS], bf16, name="tp", tag="tp")
                    nc.tensor.transpose(
                        tp[:, :], wB[:, ch * LH : (ch + 1) * LH, k], ident[:, :]
                    )
                    nc.scalar.copy(out=wTs[:, ch, k, :], in_=tp[:, :])

            def aw_phase(ps2):
                """capRaw(PSUM)[b, k, d] = sum_{l,d'} w[b,l,k] seq[b,l,d']
                W[l, kD+d, d'].  Aw = w*seq formed on DVE/Pool in l-partition
                layout; PE contracts (l, d') into 4 per-k PSUM chains."""
                nch = D // NDC
                for ci in range(nch):
                    dc = ci * NDC
                    w5c = w5p.tile([LH, 2, NDC, E], bf16, name="w5c", tag="w5")
                    # alternate DMA queues (SP / ACT) to halve the W5 stream time
                    (nc.sync if ci % 2 == 0 else nc.scalar).dma_start(
                        out=w5c[:], in_=w5_d[:, :, dc : dc + NDC, :]
                    )
                    aw = awp.tile([LH, 2, NDC, K, BS], bf16, name="aw", tag="aw")
                    eng = nc.gpsimd if ci in AW_POOL else nc.vector
                    for ch in (0, 1):
                        seqin = bass.AP(
                            tensor=seqL.tensor,
                            offset=seqL.offset + (ch * D + dc) * BS,
                            ap=[seqL.ap[0], [BS, NDC], [0, K], [1, BS]],
                        )
                        win = bass.AP(
                            tensor=wTs.tensor,
                            offset=wTs.offset + ch * K * BS,
                            ap=[wTs.ap[0], [0, NDC], [BS, K], [1, BS]],
                        )
                        eng.tensor_tensor(
                            out=aw[:, ch], in0=seqin, in1=win, op=OP.mult
                        )
                    for ch in (0, 1):
                        for j in range(NDC):
                            for k in range(K):
                                nc.tensor.matmul(
                                    ps2[:, k, :],
                                    lhsT=aw[:, ch, j, k, :],
                                    rhs=w5c[:, ch, j, k * D : (k + 1) * D],
                                    # ps2 is 1KB/partition: all four k chains
                                    # share one 2KB PSUM zero region, so only
                                    # the very first matmul may set start.
                                    start=(
                                        ci == 0 and ch == 0 and j == 0 and k == 0
                                    ),
                                    stop=(
                                        ci == nch - 1 and ch == 1 and j == NDC - 1
                                    ),
                                    skip_group_check=True,
                                )

            def squash_psum(ps2, final):
                """squash scalars from the PSUM capRaw; emit capB (pre-scaled
                cap for the next delta) or the final output."""
                capF = scr.tile([BS, K, D], f32, name="capF", tag="capF")
                nc.vector.tensor_copy(out=capF[:], in_=ps2[:])
                for k in range(K):
                    u2 = scr.tile([BS, D], f32, name="u2", tag="u2")
                    nc.vector.scalar_tensor_tensor(
                        out=u2[:],
                        in0=capF[:, k, :],
                        scalar=1.0,
                        in1=capF[:, k, :],
                        op0=OP.mult,
                        op1=OP.mult,
                        accum_out=nvec[:, k : k + 1],
                    )
                nc.scalar.activation(out=lnt, in_=nvec, func=AF.Ln, bias=epsB[:])
                nc.scalar.activation(out=rt, in_=lnt, func=AF.Exp, scale=0.5)
                nc.vector.tensor_scalar_add(out=np1, in0=nvec, scalar1=1.0)
                nc.vector.tensor_mul(out=den, in0=np1, in1=rt)
                nc.vector.reciprocal(out=dinv, in_=den)
                nc.vector.tensor_mul(out=svec, in0=nvec, in1=dinv)
                for k in range(K):
                    if final:
                        nc.vector.tensor_scalar_mul(
                            out=capOut[:, k * D : (k + 1) * D],
                            in0=capF[:, k, :],
                            scalar1=svec[:, k : k + 1],
                        )
                    else:
                        nc.vector.tensor_scalar_mul(
                            out=capB[:, :, k],
                            in0=capF[:, k, :],
                            scalar1=svec[:, k : k + 1],
                        )

            def delta_cw_chunk(l0, nl):
                """cw[:, l0:l0+nl, :] += sum_d hat[b,l,d,k]*capB[b,d,k].
                capB is pre-scaled by svec (squash), so this IS the cw update.
                Column-split DVE/Pool; both partials added into cw."""
                parts = []
                for eng, d0, d1, tag in (
                    (nc.vector, 0, DS_DEL, "ud_d"),
                    (nc.gpsimd, DS_DEL, D, "ud_p"),
                ):
                    nd = d1 - d0
                    u = scr.tile([BS, PB, nd * K], bf16, name=tag, tag=tag)
                    cin = bass.AP(
                        tensor=capB.tensor,
                        offset=capB.offset + d0 * K,
                        ap=[capB.ap[0], [0, nl], [1, nd * K]],
                    )
                    eng.tensor_tensor(
                        out=u[:, 0:nl, :],
                        in0=hat[:, l0 : l0 + nl, d0 * K : d1 * K],
                        in1=cin,
                        op=OP.mult,
                    )
                    fold_d(eng, u, nl, nd)
                    parts.append(u)
                for u in parts:
                    nc.vector.tensor_add(
                        out=cw[:, l0 : l0 + nl, :],
                        in0=cw[:, l0 : l0 + nl, :],
                        in1=u[:, 0:nl, 0:K],
                    )

            def softmax_chunk(l0, nl):
                """wB[:, l0:l0+nl, :] = softmax_k(cw) in bf16. w holds exp(cw);
                the normalize + bf16 downcast are fused into one mult."""
                nc.scalar.activation(
                    out=w[:, l0 : l0 + nl, :], in_=cw[:, l0 : l0 + nl, :], func=AF.Exp
                )
                nc.vector.tensor_reduce(
                    out=zsum[:, l0 : l0 + nl],
                    in_=w[:, l0 : l0 + nl, :],
                    axis=mybir.AxisListType.X,
                    op=OP.add,
                )
                nc.vector.reciprocal(
                    out=zinv[:, l0 : l0 + nl], in_=zsum[:, l0 : l0 + nl]
                )
                zin = bass.AP(
                    tensor=zinv.tensor,
                    offset=zinv.offset + l0,
                    ap=[zinv.ap[0], [1, nl], [0, K]],
                )
                nc.vector.tensor_tensor(
                    out=wB[:, l0 : l0 + nl, :],
                    in0=w[:, l0 : l0 + nl, :],
                    in1=zin,
                    op=OP.mult,
                )

            def squash_scalars():
                for k in range(K):
                    u2 = scr.tile([BS, D], f32, name="u2", tag="u2")
                    nc.vector.scalar_tensor_tensor(
                        out=u2[:],
                        in0=capRaw[:, :, k],
                        scalar=1.0,
                        in1=capRaw[:, :, k],
                        op0=OP.mult,
                        op1=OP.mult,
                        accum_out=nvec[:, k : k + 1],
                    )
                nc.scalar.activation(out=lnt, in_=nvec, func=AF.Ln, bias=epsB[:])
                nc.scalar.activation(out=rt, in_=lnt, func=AF.Exp, scale=0.5)
                nc.vector.tensor_scalar_add(out=np1, in0=nvec, scalar1=1.0)
                nc.vector.tensor_mul(out=den, in0=np1, in1=rt)
                nc.vector.reciprocal(out=dinv, in_=den)
                nc.vector.tensor_mul(out=svec, in0=nvec, in1=dinv)
                # capB = svec * capRaw (pre-scaled by the squash factor, so the
                # delta contraction directly yields the cw increment)
                sin = bass.AP(
                    tensor=smalls.tensor,
                    offset=smalls.offset + 6 * K,
                    ap=[smalls.ap[0], [0, D], [1, K]],
                )
                nc.vector.tensor_tensor(
                    out=capB[:], in0=capRaw[:], in1=sin, op=OP.mult
                )

            # ================= phase 1: hat einsum + first capacc =================
            softmax_full()
            nc.vector.memset(capRaw[:], 0.0)

            for ci in range(NMC):
                mc = ci * MC
                nm = min(MC, M - mc)
                seqc = seqp.tile([128, MC, BS], bf16, name="seqc", tag="seqc")
                wc = wtp.tile([128, MC, E], bf16, name="wc", tag="wc")
                nc.sync.dma_start(
                    out=seqc[:, 0:nm, :], in_=seqT_d[:, mc : mc + nm, :]
                )
                nc.sync.dma_start(out=wc[:, 0:nm, :], in_=wT_d[:, mc : mc + nm, :])
                # parity-grouped matmuls: all even l's of the chunk, then odd
                for par in (0, 1):
                    p0 = 64 * par
                    for b0 in range(0, nm, 4):
                        nb = min(4, nm - b0)
                        ps = psum.tile([128, 4, E], f32, name="ps", tag="ps")
                        for j in range(nb):
                            mi = b0 + j
                            nc.tensor.matmul(
                                ps[:, j, :],
                                lhsT=seqc[p0 : p0 + 64, mi, :],
                                rhs=wc[p0 : p0 + 64, mi, :],
                                start=(j % LPB == 0),
                                stop=(j % LPB == LPB - 1 or j == nb - 1),
                                skip_group_check=True,
                            )
                        # copy to hat rows l = 2*(mc+b0+j) + par, split ACT/Pool
                        hout_a = bass.AP(
                            tensor=hat.tensor,
                            offset=hat.offset + (2 * (mc + b0) + par) * E,
                            ap=[hat.ap[0], [2 * E, nb], [1, DS_CP * K]],
                        )
                        hout_p = bass.AP(
                            tensor=hat.tensor,
                            offset=hat.offset + (2 * (mc + b0) + par) * E + DS_CP * K,
                            ap=[hat.ap[0], [2 * E, nb], [1, (D - DS_CP) * K]],
                        )
                        nc.scalar.copy(out=hout_a, in_=ps[:, 0:nb, 0 : DS_CP * K])
                        nc.vector.tensor_copy(
                            out=hout_p, in_=ps[:, 0:nb, DS_CP * K : E]
                        )
                # first-iteration capacc on the now-complete contiguous l range
                capacc_chunk(2 * mc, 2 * nm)

            nc.sync.dma_start(out=seqL[:], in_=seqL_d[:])
            squash_scalars()

            # ================= streams 2/3 =================
            if DEBUG:
                nc.sync.dma_start(out=dbg1_d[:], in_=capB[:])

            for it in (1, 2):
                for l0 in range(0, L, PB):
                    nl = min(PB, L - l0)
                    delta_cw_chunk(l0, nl)
                    softmax_chunk(l0, nl)
                    if l0 + nl in (LH, L):
                        transposes_half((l0 + nl) // LH - 1)
                if DEBUG and it == 1:
                    nc.sync.dma_start(out=dbg3_d[:], in_=wTs[:])
                ps2 = capps.tile([BS, K, D], f32, name="ps2", tag="ps2")
                aw_phase(ps2)
                if DEBUG and it == 1:
                    nc.vector.tensor_copy(
                        out=capOut.rearrange("b (k d) -> b k d", k=K), in_=ps2[:]
                    )
                    nc.sync.dma_start(out=dbg2_d[:], in_=capOut[:])
                squash_psum(ps2, final=(it == 2))

            nc.sync.dma_start(out=out_d[:], in_=capOut[:])

    nc.finalize()
    return nc


_NC_CACHE = None


def _get_nc():
    global _NC_CACHE
    if _NC_CACHE is None:
        _NC_CACHE = build_nc()
    return _NC_CACHE


def prep_inputs(seq_out, weights, capsule_weight):
    """Host-side layout prep -> list of per-core input maps."""
    import ml_dtypes

    bf16 = ml_dtypes.bfloat16
    seq = np.ascontiguousarray(np.asarray(seq_out, dtype=np.float32))
    W = np.ascontiguousarray(np.asarray(weights, dtype=np.float32))[0]  # [L,E,D]
    cwf = np.ascontiguousarray(np.asarray(capsule_weight, dtype=np.float32))

    # seqT[p, m, b] = seq[b, 2m + p//64, p%64]
    seqT = np.ascontiguousarray(
        seq.reshape(B, M, 2, D).transpose(2, 3, 1, 0).reshape(128, M, B).astype(bf16)
    )
    # wT[p, m, (d,k)] = W[2m + p//64, k*D + d, p%64]   (hat free axis = (d,k))
    wTf = W.reshape(M, 2, K, D, D).transpose(1, 4, 0, 3, 2)  # [par, d', m, d, k]
    wT = np.ascontiguousarray(wTf.reshape(128, M, E).astype(bf16))
    # cwA[b, l, k] = cw[b, k, l]
    cwA = np.ascontiguousarray(cwf.transpose(0, 2, 1))  # [B, L, K]
    # seqL[lp, ch, d', b] = seq[b, ch*LH+lp, d']
    seqL = np.ascontiguousarray(
        seq.reshape(B, 2, LH, D).transpose(2, 1, 3, 0).astype(bf16)
    )
    # w5[lp, ch, d', e] = W[ch*LH+lp, e, d']
    w5 = np.ascontiguousarray(
        W.reshape(2, LH, E, D).transpose(1, 0, 3, 2).astype(bf16)
    )

    in_maps = []
    for c in range(NCORES):
        in_maps.append(
            {
                "seqT": np.ascontiguousarray(seqT[:, :, c * BS : (c + 1) * BS]),
                "wT": wT,
                "seqL": np.ascontiguousarray(seqL[:, :, :, c * BS : (c + 1) * BS]),
                "w5": w5,
                "cw": np.ascontiguousarray(cwA[c * BS : (c + 1) * BS]),
            }
        )
    return in_maps


def gather_out(results):
    """Per-core 'out' [BS, E=(k*D+d)] -> full [B, K, D]."""
    return np.concatenate(
        [r["out"].reshape(BS, K, D) for r in results], axis=0
    ).astype(np.float32)


def kernel(seq_out, mask, weights, capsule_weight):
    from concourse.bass_utils import run_bass_kernel_spmd

    nc = _get_nc()
    in_maps = prep_inputs(seq_out, weights, capsule_weight)
    res = run_bass_kernel_spmd(nc, in_maps, core_ids=list(range(NCORES)))
    return gather_out(res.results)


if __name__ == "__main__":
    rng = np.random.default_rng(0)
    seq_out = rng.standard_normal((B, L, D), dtype=np.float32)
    mask = np.ones((B, L), dtype=np.float32)
    weights = (0.02 * rng.standard_normal((1, L, E, D))).astype(np.float32)
    capsule_weight = rng.standard_normal((B, K, L), dtype=np.float32)
    out = kernel(seq_out, mask, weights, capsule_weight)
    print("out", out.shape, out.dtype, float(np.abs(out).max()))
